# revision 12
# baseline (speedup 1.0000x reference)
"""Trainium2 Bass kernel for the Complex_Linear_DS problem.

Reference computation:
    h  = relu(target @ W1.T + b1)                  # (1024, 256)
    x0 = (h @ W2.T + b2).T                         # (256, 1024)
    repeat 800x: act_t = tanh(x_t.T @ Wm.T + bm);  x_{t+1} = x_t - (A @ x_t) * dt
    outputs: actions (1024, 2, 800), hidden_states (800, 256, 1024)

Strategy (8 NeuronCores):
  hidden[t] = M^t @ x0 with M = I - dt*A.  Shard TIME across the 8 cores:
  core c owns t in [100c, 100c+100).  The host precomputes (in float64) the
  per-core jump matrix Kc = M^(100c) plus the in-core power basis
  P_j = M^j (j=1..20), so each core evaluates its 100 states as
      x_seed = Kc @ x0                  (device, fp32 matmul)
      chunk seeds s_i = P_20 @ s_{i-1}  (chain depth <= 5)
      states     x = P_j @ s_i          (one fp32r matmul per state)
  i.e. every state is produced by a single (256,256)@(256,1024) matmul in
  float32r (fp22-truncated reads, 1 cycle/row on the PE) with multiplicative
  truncation bias bounded by <= 6 applications (~5e-4), not 800.
  Actions come from host-precomputed V rows (Wm @ M^tau) applied to the seed,
  tanh'd on ScalarE, transposed on the PE.
  Per-core output: hidden slab (100,256,1024) = 100 MiB streamed over HBM.
"""

import numpy as np

T_STEP = 0.01
N_STEPS = 800
N_TGT = 1024
H = 256
INPUT_S = 2
ACTION_S = 2
NCORES = 8
TL = N_STEPS // NCORES          # 100 local steps per core
KS = 20                         # power-basis stride
NCH = TL // KS                  # 5 chunks per core

_PROGRAM_CACHE = {}


def _build_program():
    """Build (once) the SPMD Bass program shared by all 8 cores."""
    if "nc" in _PROGRAM_CACHE:
        return _PROGRAM_CACHE["nc"]

    from contextlib import ExitStack

    import concourse.bacc as bacc
    import concourse.bass as bass
    import concourse.mybir as mybir
    import concourse.tile as tile

    f32 = mybir.dt.float32
    f32r = mybir.dt.float32r
    AF = mybir.ActivationFunctionType

    nc = bacc.Bacc("TRN2", target_bir_lowering=False, debug=False)

    # ---- I/O -------------------------------------------------------------
    tgtT = nc.dram_tensor("tgtT", [INPUT_S, N_TGT], f32, kind="ExternalInput").ap()
    w1T = nc.dram_tensor("w1T", [INPUT_S, H], f32, kind="ExternalInput").ap()
    b1c = nc.dram_tensor("b1c", [128, 2], f32, kind="ExternalInput").ap()
    w2s = nc.dram_tensor("w2s", [128, 2, 2, 128], f32, kind="ExternalInput").ap()
    b2c = nc.dram_tensor("b2c", [128, 2], f32, kind="ExternalInput").ap()
    kcs = nc.dram_tensor("kcs", [128, 2, 2, 128], f32, kind="ExternalInput").ap()
    pts = nc.dram_tensor("pts", [128, KS, 2, 2, 128], f32r, kind="ExternalInput").ap()
    vts = nc.dram_tensor("vts", [128, 2, 2, TL], f32, kind="ExternalInput").ap()
    bmv = nc.dram_tensor("bmv", [TL, 1], f32, kind="ExternalInput").ap()
    iden = nc.dram_tensor("iden", [128, 128], f32, kind="ExternalInput").ap()

    hid = nc.dram_tensor("hidden", [TL, H, N_TGT], f32, kind="ExternalOutput").ap()
    act = nc.dram_tensor("actions", [N_TGT, ACTION_S, TL], f32, kind="ExternalOutput").ap()

    with tile.TileContext(nc) as tc, ExitStack() as ctx:
        const = ctx.enter_context(tc.tile_pool(name="const", bufs=1))
        apool = ctx.enter_context(tc.tile_pool(name="acts", bufs=2))
        xpool = ctx.enter_context(tc.tile_pool(name="x", bufs=8))
        ps = ctx.enter_context(
            tc.tile_pool(name="ps", bufs=2, space=bass.MemorySpace.PSUM)
        )

        # ---- weight / constant loads ------------------------------------
        tgtT_sb = const.tile([INPUT_S, N_TGT], f32)
        nc.sync.dma_start(tgtT_sb[:], tgtT[:])
        w1T_sb = const.tile([INPUT_S, H], f32)
        nc.sync.dma_start(w1T_sb[:], w1T[:])
        b1_sb = const.tile([128, 2], f32)
        nc.sync.dma_start(b1_sb[:], b1c[:])
        w2_sb = const.tile([128, 2, 2, 128], f32)
        nc.sync.dma_start(w2_sb[:], w2s[:])
        b2_sb = const.tile([128, 2], f32)
        nc.sync.dma_start(b2_sb[:], b2c[:])
        kc_sb = const.tile([128, 2, 2, 128], f32)
        nc.sync.dma_start(kc_sb[:], kcs[:])
        pt_sb = const.tile([128, KS, 2, 2, 128], f32r)
        nc.sync.dma_start(pt_sb[:], pts[:])
        vt_sb = const.tile([128, 2, 2, TL], f32)
        nc.sync.dma_start(vt_sb[:], vts[:])
        bm_sb = const.tile([TL, 1], f32)
        nc.sync.dma_start(bm_sb[:], bmv[:])
        id_sb = const.tile([128, 128], f32)
        nc.sync.dma_start(id_sb[:], iden[:])

        # ---- prologue: h = relu(W1 @ tgtT + b1) -------------------------
        ph = ps.tile([128, 2, 1024], f32, tag="ps")
        for mi in range(2):
            for nh in range(2):
                nc.tensor.matmul(
                    ph[:, mi, nh * 512 : (nh + 1) * 512],
                    w1T_sb[:, mi * 128 : (mi + 1) * 128],
                    tgtT_sb[:, nh * 512 : (nh + 1) * 512],
                    start=True,
                    stop=True,
                )
        h_sb = const.tile([128, 2, 1024], f32)
        for mi in range(2):
            nc.scalar.activation(
                h_sb[:, mi, :], ph[:, mi, :], AF.Relu, bias=b1_sb[:, mi : mi + 1]
            )

        # ---- x0 = W2 @ h + b2 -------------------------------------------
        px = ps.tile([128, 2, 1024], f32, tag="ps")
        for mi in range(2):
            for nh in range(2):
                for ki in range(2):
                    nc.tensor.matmul(
                        px[:, mi, nh * 512 : (nh + 1) * 512],
                        w2_sb[:, ki, mi, :],
                        h_sb[:, ki, nh * 512 : (nh + 1) * 512],
                        start=(ki == 0),
                        stop=(ki == 1),
                    )
        x0_sb = const.tile([128, 2, 1024], f32)
        for mi in range(2):
            nc.scalar.activation(
                x0_sb[:, mi, :], px[:, mi, :], AF.Identity, bias=b2_sb[:, mi : mi + 1]
            )

        # ---- seed = Kc @ x0 ---------------------------------------------
        psd = ps.tile([128, 2, 1024], f32, tag="ps")
        for mi in range(2):
            for nh in range(2):
                for ki in range(2):
                    nc.tensor.matmul(
                        psd[:, mi, nh * 512 : (nh + 1) * 512],
                        kc_sb[:, ki, mi, :],
                        x0_sb[:, ki, nh * 512 : (nh + 1) * 512],
                        start=(ki == 0),
                        stop=(ki == 1),
                    )
        # two seed copies: fp22-rounded one feeds the fp32r main loop; the
        # unrounded one feeds the (full fp32) action matmuls + hidden[0]
        seed = const.tile([128, 2, 1024], f32)
        nc.vector.tensor_copy(seed[:, 0, :].bitcast(f32r), psd[:, 0, :])
        nc.scalar.copy(seed[:, 1, :].bitcast(f32r), psd[:, 1, :])
        seed_f = const.tile([128, 2, 1024], f32)
        nc.vector.tensor_copy(seed_f[:, 0, :], psd[:, 0, :])
        nc.scalar.copy(seed_f[:, 1, :], psd[:, 1, :])
        nc.sync.dma_start(hid[0].rearrange("(mi p) n -> p mi n", p=128), seed_f[:])

        # ---- actions: AA_v = tanh(VV_v @ seed + bm) then transpose ------
        acts_sb = const.tile([128, 8, 2 * TL], f32)
        for v in range(2):
            pa = ps.tile([128, 2, 1024], f32, tag="ps")
            for nh in range(2):
                for ki in range(2):
                    nc.tensor.matmul(
                        pa[0:TL, 0, nh * 512 : (nh + 1) * 512],
                        vt_sb[:, v, ki, :],
                        seed_f[:, ki, nh * 512 : (nh + 1) * 512],
                        start=(ki == 0),
                        stop=(ki == 1),
                    )
            aa = apool.tile([TL, N_TGT], f32, tag="aa")
            nc.scalar.activation(aa[:], pa[0:TL, 0, :], AF.Tanh, bias=bm_sb[:])
            for tb in range(8):
                pt = ps.tile([128, 2, 1024], f32, tag="ps")
                nc.tensor.transpose(
                    pt[:, 0, 0:TL],
                    aa[:, tb * 128 : (tb + 1) * 128],
                    id_sb[0:TL, 0:TL],
                )
                half = KS * NCH // 2  # 50
                nc.vector.tensor_copy(
                    acts_sb[:, tb, v * half : (v + 1) * half], pt[:, 0, 0:half]
                )
                nc.vector.tensor_copy(
                    acts_sb[:, tb, TL + v * half : TL + (v + 1) * half],
                    pt[:, 0, half : 2 * half],
                )
        act_r = act.rearrange("(tb p) a t -> tb p (a t)", p=128)
        for tb in range(8):
            nc.sync.dma_start(act_r[tb], acts_sb[:, tb, :])

        # ---- main loop: 99 states, each one fp32r matmul ----------------
        cur = seed
        for i in range(NCH):
            js = ([KS] + list(range(1, KS))) if i < NCH - 1 else list(range(1, KS))
            nxt = None
            for j in js:
                t_loc = KS * i + j
                pj = ps.tile([128, 2, 1024], f32, tag="ps")
                for mi in range(2):
                    for nh in range(2):
                        for ki in range(2):
                            nc.tensor.matmul(
                                pj[:, mi, nh * 512 : (nh + 1) * 512],
                                pt_sb[:, j - 1, ki, mi, :],
                                cur[:, ki, nh * 512 : (nh + 1) * 512].bitcast(f32r),
                                start=(ki == 0),
                                stop=(ki == 1),
                            )
                xt = xpool.tile([128, 2, 1024], f32, tag="xt")
                if j == KS:
                    # chain tile feeds the next chunk's fp32r matmuls: the
                    # producing copies must emit fp32r-rounded values
                    nc.vector.tensor_copy(xt[:, 0, :].bitcast(f32r), pj[:, 0, :])
                    nc.scalar.copy(xt[:, 1, :].bitcast(f32r), pj[:, 1, :])
                else:
                    nc.vector.tensor_copy(xt[:, 0, :], pj[:, 0, :])
                    nc.scalar.copy(xt[:, 1, :], pj[:, 1, :])
                nc.sync.dma_start(
                    hid[t_loc].rearrange("(mi p) n -> p mi n", p=128), xt[:]
                )
                if j == KS:
                    nxt = xt
            if nxt is not None:
                cur = nxt

    nc.compile()
    _PROGRAM_CACHE["nc"] = nc
    return nc


def _host_inputs(target, A, W1, b1, W2, b2, Wm, bm):
    """Host-side (float64) preprocessing: jump matrices, power basis, V rows."""
    A64 = np.asarray(A, dtype=np.float64)
    M64 = np.eye(H, dtype=np.float64) - T_STEP * A64

    # P_j = M^j, j = 1..KS
    P = np.empty((KS, H, H), dtype=np.float64)
    P[0] = M64
    for j in range(1, KS):
        P[j] = P[j - 1] @ M64

    # per-core jump Kc = M^(TL*c)
    MTL = np.linalg.matrix_power(M64, TL)
    Ks = np.empty((NCORES, H, H), dtype=np.float64)
    Ks[0] = np.eye(H, dtype=np.float64)
    for c in range(1, NCORES):
        Ks[c] = Ks[c - 1] @ MTL

    # V_tau = Wm @ M^tau, tau = 0..TL-1
    Wm64 = np.asarray(Wm, dtype=np.float64)
    Vt = np.empty((TL, ACTION_S, H), dtype=np.float64)
    Vt[0] = Wm64
    for tau in range(1, TL):
        Vt[tau] = Vt[tau - 1] @ M64

    half = TL // 2  # 50
    # VV[v, a*half+jj, :] = Vt[half*v + jj][a]
    VV = np.empty((2, 2 * half, H), dtype=np.float64)
    for v in range(2):
        for a in range(ACTION_S):
            VV[v, a * half : (a + 1) * half, :] = Vt[half * v : half * (v + 1), a, :]

    f32 = np.float32

    def round_fp22(x):
        # round-to-nearest onto the e8m13 grid the PE's float32r reads use,
        # so the hardware's truncate-to-FP22 is exact (no downward bias)
        u = np.asarray(x, f32).view(np.uint32)
        u = (u + np.uint32(0x200)) & np.uint32(0xFFFFFC00)
        return u.view(f32)

    def swz_mat(X):  # (256,256), out[kp,ki,mi,m] = X[mi*128+m, ki*128+kp]
        return np.ascontiguousarray(
            X.reshape(2, 128, 2, 128).transpose(3, 2, 0, 1).astype(f32)
        )

    pts_np = np.ascontiguousarray(
        round_fp22(P.reshape(KS, 2, 128, 2, 128).transpose(4, 0, 3, 1, 2).astype(f32))
    )  # [kp, j, ki, mi, m] = P_j[mi*128+m, ki*128+kp]
    vts_np = np.ascontiguousarray(
        VV.reshape(2, 2 * half, 2, 128).transpose(3, 0, 2, 1).astype(f32)
    )  # [kp, v, ki, r] = VV[v, r, ki*128+kp]

    common = {
        "tgtT": np.ascontiguousarray(np.asarray(target, f32).T),
        "w1T": np.ascontiguousarray(np.asarray(W1, f32).T),
        "b1c": np.ascontiguousarray(np.asarray(b1, f32).reshape(2, 128).T),
        "w2s": swz_mat(np.asarray(W2, np.float64)),
        "b2c": np.ascontiguousarray(np.asarray(b2, f32).reshape(2, 128).T),
        "pts": pts_np,
        "vts": vts_np,
        "bmv": np.ascontiguousarray(
            np.repeat(np.asarray(bm, f32), half).reshape(TL, 1)
        ),
        "iden": np.eye(128, dtype=f32),
    }
    in_maps = []
    for c in range(NCORES):
        m = dict(common)
        m["kcs"] = swz_mat(Ks[c])
        in_maps.append(m)
    return in_maps


def kernel(target, A, W1, b1, W2, b2, Wm, bm, _trace=False):
    from concourse.bass_utils import run_bass_kernel_spmd

    nc = _build_program()
    in_maps = _host_inputs(target, A, W1, b1, W2, b2, Wm, bm)
    res = run_bass_kernel_spmd(
        nc, in_maps, list(range(NCORES)), trace=_trace
    )
    hidden = np.concatenate([res.results[c]["hidden"] for c in range(NCORES)], axis=0)
    actions = np.concatenate([res.results[c]["actions"] for c in range(NCORES)], axis=2)
    if _trace:
        kernel.last_exec_time_ns = res.exec_time_ns
        kernel.last_results = res
    return actions, hidden
